# Initial kernel scaffold
#
"""Trainium2 Bass kernel: 4096x4096 fp32 'valid' cross-correlation with a 15x15
kernel, plus scalar bias.

Strategy
--------
- Shard output rows (H) across 8 NeuronCores: each core computes 512 output
  rows (core 7's tail rows are trimmed on the host). Each core's input slice is
  its 512 rows plus a 14-row halo (gathered on the host; inputs overlap, no
  device-to-device communication needed).
- On each core the 2D conv is computed as a sum of 15 banded-Toeplitz matmuls
  accumulated in PSUM: for each kernel column dj, a [K=128, M=114] Toeplitz
  matrix T_dj (T_dj[k, m] = weight[k-m, dj]) contracts 128 input rows against
  114 output rows; the W-shift for dj is absorbed as a free-dim offset in the
  moving operand (the image rows live in SBUF partitions, W along the free
  dim). All Toeplitz matrices are built on the host from the runtime weight.
- Matmuls run in float32r (TF32-like, ~1e-4 rel err) at 1 cycle/row -- 4x
  faster than plain fp32 matmul.
"""

import numpy as np

H, W = 4096, 4096
KH, KW = 15, 15
HO, WO = H - KH + 1, W - KW + 1  # 4082, 4082
NCORES = 8
R = 512              # output rows computed per core (uniform SPMD shape)
RIN = R + KH - 1     # input rows per core slice (with halo)
MCH = 114            # output rows per h-chunk (114 + 14 = 128 = contraction K)
NCH = 512            # output cols per w-chunk (one PSUM bank, fp32 moving max)

# h-chunks: (m0, Mc, K)
H_CHUNKS = [(b * MCH, min(MCH, R - b * MCH), min(MCH, R - b * MCH) + KH - 1)
            for b in range((R + MCH - 1) // MCH)]
# w-chunks: (n0, Nc)
W_CHUNKS = [(n0, min(NCH, WO - n0)) for n0 in range(0, WO, NCH)]

_CACHE = {}


def _build_nc():
    import concourse.bacc as bacc
    import concourse.mybir as mybir
    from concourse.tile import TileContext

    f32 = mybir.dt.float32
    f32r = mybir.dt.float32r

    nc = bacc.Bacc("TRN2", debug=False, num_devices=NCORES)
    xs_d = nc.dram_tensor("xs", [RIN, W], f32, kind="ExternalInput")
    wT_d = nc.dram_tensor("wT", [128, KW, MCH], f32, kind="ExternalInput")
    bias_d = nc.dram_tensor("bias", [1, 1], f32, kind="ExternalInput")
    ys_d = nc.dram_tensor("ys", [R, WO], f32, kind="ExternalOutput")

    with TileContext(nc) as tc:
        with (
            tc.tile_pool(name="xp", bufs=1) as xp,
            tc.tile_pool(name="wp", bufs=1) as wp,
            tc.tile_pool(name="op", bufs=4) as op,
            tc.tile_pool(name="pp", bufs=4, space="PSUM") as pp,
        ):
            # Weights (Toeplitz stack) + bias
            w_t = wp.tile([128, KW, MCH], f32r)
            nc.sync.dma_start(w_t[:, :, :], wT_d[:, :, :].bitcast(f32r))
            bias_t = wp.tile([1, 1], f32)
            nc.sync.dma_start(bias_t[:, :], bias_d[:, :])
            bias_bc = wp.tile([128, 1], f32)
            nc.gpsimd.partition_broadcast(bias_bc[:, :], bias_t[:, :])

            # Input row-windows, one per h-chunk, DMA'd in column slices
            x_tiles = []
            for b, (m0, Mc, K) in enumerate(H_CHUNKS):
                x_b = xp.tile([K, W], f32r, name=f"x{b}")
                for c0 in range(0, W, 1024):
                    nc.sync.dma_start(
                        x_b[:, c0:c0 + 1024],
                        xs_d[m0:m0 + K, c0:c0 + 1024].bitcast(f32r),
                    )
                x_tiles.append(x_b)

            for b, (m0, Mc, K) in enumerate(H_CHUNKS):
                x_b = x_tiles[b]
                for n0, Nc in W_CHUNKS:
                    ps = pp.tile([MCH, NCH], f32, name="ps")
                    for dj in range(KW):
                        nc.tensor.matmul(
                            ps[0:Mc, 0:Nc],
                            w_t[0:K, dj, 0:Mc],
                            x_b[0:K, n0 + dj:n0 + dj + Nc],
                            start=(dj == 0),
                            stop=(dj == KW - 1),
                        )
                    o = op.tile([MCH, NCH], f32, name="o")
                    nc.vector.tensor_scalar_add(
                        o[0:Mc, 0:Nc], ps[0:Mc, 0:Nc], bias_bc[0:Mc, 0:1]
                    )
                    nc.sync.dma_start(
                        ys_d[m0:m0 + Mc, n0:n0 + Nc], o[0:Mc, 0:Nc]
                    )

    nc.compile()
    return nc


def _toeplitz_stack(weight: np.ndarray) -> np.ndarray:
    """wT[k, dj, m] = weight[k-m, dj] for 0 <= k-m < KH."""
    wT = np.zeros((128, KW, MCH), dtype=np.float32)
    for di in range(KH):
        for m in range(MCH):
            wT[m + di, :, m] = weight[di, :]
    return wT


def kernel(x: np.ndarray, weight: np.ndarray, bias: np.ndarray) -> np.ndarray:
    from concourse.bass_utils import run_bass_kernel_spmd

    if "nc" not in _CACHE:
        _CACHE["nc"] = _build_nc()
    nc = _CACHE["nc"]

    x = np.ascontiguousarray(x, dtype=np.float32)
    weight = np.asarray(weight, dtype=np.float32)
    bias_v = np.asarray(bias, dtype=np.float32).reshape(-1)[:1]

    x_pad = np.zeros((NCORES * R + KH - 1, W), dtype=np.float32)
    x_pad[:H] = x
    wT = _toeplitz_stack(weight)
    bias_in = bias_v.reshape(1, 1)

    in_maps = [
        {"xs": x_pad[c * R:c * R + RIN], "wT": wT, "bias": bias_in}
        for c in range(NCORES)
    ]
    res = run_bass_kernel_spmd(nc, in_maps, core_ids=list(range(NCORES)))

    out = np.empty((HO, WO), dtype=np.float32)
    for c in range(NCORES):
        r0 = c * R
        r1 = min(r0 + R, HO)
        out[r0:r1] = res.results[c]["ys"][: r1 - r0]
    return out


# revision 2
# speedup vs baseline: 1.2485x; 1.2485x over previous
"""Trainium2 Bass kernel: 4096x4096 fp32 'valid' cross-correlation with a 15x15
kernel, plus scalar bias.

Strategy
--------
- Shard output rows (H) across 8 NeuronCores: each core computes 512 output
  rows (core 7's tail rows are trimmed on the host). Each core's input slice is
  its 512 rows plus a 14-row halo (gathered on the host; inputs overlap, no
  device-to-device communication needed).
- On each core the 2D conv is computed as a sum of 15 banded-Toeplitz matmuls
  accumulated in PSUM: for each kernel column dj, a [K=128, M=114] Toeplitz
  matrix T_dj (T_dj[k, m] = weight[k-m, dj]) contracts 128 input rows against
  114 output rows; the W-shift for dj is absorbed as a free-dim offset in the
  moving operand (the image rows live in SBUF partitions, W along the free
  dim). All Toeplitz matrices are built on the host from the runtime weight.
- Matmuls run in float32r (TF32-like, ~1e-4 rel err) at 1 cycle/row -- 4x
  faster than plain fp32 matmul.
"""

import numpy as np

H, W = 4096, 4096
KH, KW = 15, 15
HO, WO = H - KH + 1, W - KW + 1  # 4082, 4082
NCORES = 8
R = 512              # output rows computed per core (uniform SPMD shape)
RIN = R + KH - 1     # input rows per core slice (with halo)
MCH = 114            # output rows per h-chunk (114 + 14 = 128 = contraction K)
NCH = 512            # output cols per w-chunk (one PSUM bank, fp32 moving max)

# h-chunks: (m0, Mc, K)
H_CHUNKS = [(b * MCH, min(MCH, R - b * MCH), min(MCH, R - b * MCH) + KH - 1)
            for b in range((R + MCH - 1) // MCH)]
# w-chunks: (n0, Nc)
W_CHUNKS = [(n0, min(NCH, WO - n0)) for n0 in range(0, WO, NCH)]

_CACHE = {}


def _build_nc(reps: int = 1):
    import concourse.bacc as bacc
    import concourse.mybir as mybir
    from concourse.tile import TileContext

    f32 = mybir.dt.float32
    f32r = mybir.dt.float32r

    nc = bacc.Bacc("TRN2", debug=False, num_devices=NCORES)
    xs_d = nc.dram_tensor("xs", [RIN, W], f32, kind="ExternalInput")
    wT_d = nc.dram_tensor("wT", [128, KW, MCH], f32, kind="ExternalInput")
    bias_d = nc.dram_tensor("bias", [1, 1], f32, kind="ExternalInput")
    ys_d = nc.dram_tensor("ys", [R, WO], f32, kind="ExternalOutput")

    with TileContext(nc) as tc:
        with (
            tc.tile_pool(name="xp", bufs=1) as xp,
            tc.tile_pool(name="wp", bufs=1) as wp,
            tc.tile_pool(name="op", bufs=4) as op,
            tc.tile_pool(name="pp", bufs=4, space="PSUM") as pp,
        ):
            # Weights (Toeplitz stack) + bias
            w_t = wp.tile([128, KW, MCH], f32r)
            nc.sync.dma_start(w_t[:, :, :], wT_d[:, :, :].bitcast(f32r))
            bias_t = wp.tile([1, 1], f32)
            nc.sync.dma_start(bias_t[:, :], bias_d[:, :])
            bias_bc = wp.tile([128, 1], f32)
            nc.gpsimd.partition_broadcast(bias_bc[:, :], bias_t[:, :])

            for _rep in range(reps):
                # Input row-windows, one per h-chunk, DMA'd in column slices
                x_tiles = []
                for b, (m0, Mc, K) in enumerate(H_CHUNKS):
                    x_b = xp.tile([K, W], f32r, name=f"x{b}")
                    for c0 in range(0, W, 1024):
                        nc.sync.dma_start(
                            x_b[:, c0:c0 + 1024],
                            xs_d[m0:m0 + K, c0:c0 + 1024].bitcast(f32r),
                        )
                    x_tiles.append(x_b)

                for b, (m0, Mc, K) in enumerate(H_CHUNKS):
                    x_b = x_tiles[b]
                    for n0, Nc in W_CHUNKS:
                        ps = pp.tile([MCH, NCH], f32, name="ps")
                        for dj in range(KW):
                            nc.tensor.matmul(
                                ps[0:Mc, 0:Nc],
                                w_t[0:K, dj, 0:Mc],
                                x_b[0:K, n0 + dj:n0 + dj + Nc],
                                start=(dj == 0),
                                stop=(dj == KW - 1),
                            )
                        o = op.tile([MCH, NCH], f32, name="o")
                        nc.vector.tensor_scalar_add(
                            o[0:Mc, 0:Nc], ps[0:Mc, 0:Nc], bias_bc[0:Mc, 0:1]
                        )
                        nc.sync.dma_start(
                            ys_d[m0:m0 + Mc, n0:n0 + Nc], o[0:Mc, 0:Nc]
                        )

    nc.compile()
    return nc


def _toeplitz_stack(weight: np.ndarray) -> np.ndarray:
    """wT[k, dj, m] = weight[k-m, dj] for 0 <= k-m < KH."""
    wT = np.zeros((128, KW, MCH), dtype=np.float32)
    for di in range(KH):
        for m in range(MCH):
            wT[m + di, :, m] = weight[di, :]
    return wT


def kernel(x: np.ndarray, weight: np.ndarray, bias: np.ndarray) -> np.ndarray:
    from concourse.bass_utils import run_bass_kernel_spmd

    if "nc" not in _CACHE:
        _CACHE["nc"] = _build_nc()
    nc = _CACHE["nc"]

    x = np.ascontiguousarray(x, dtype=np.float32)
    weight = np.asarray(weight, dtype=np.float32)
    bias_v = np.asarray(bias, dtype=np.float32).reshape(-1)[:1]

    x_pad = np.zeros((NCORES * R + KH - 1, W), dtype=np.float32)
    x_pad[:H] = x
    wT = _toeplitz_stack(weight)
    bias_in = bias_v.reshape(1, 1)

    in_maps = [
        {"xs": x_pad[c * R:c * R + RIN], "wT": wT, "bias": bias_in}
        for c in range(NCORES)
    ]
    res = run_bass_kernel_spmd(nc, in_maps, core_ids=list(range(NCORES)))

    out = np.empty((HO, WO), dtype=np.float32)
    for c in range(NCORES):
        r0 = c * R
        r1 = min(r0 + R, HO)
        out[r0:r1] = res.results[c]["ys"][: r1 - r0]
    return out
